# revision 31
# baseline (speedup 1.0000x reference)
"""Trainium2 Bass kernel: multi-head attention (Graphormer-style bias+mask)
followed by a node-similarity GEMM (out = merged @ merged^T).

Sharding: pure data-parallel over batch. B=8 batch elements -> 8 NeuronCores,
one batch element per core, no collectives. Each core computes its own
[1024, 1024] output slab.

Per-core math (b fixed):
  Q^T = Wq @ x^T + bq ; K^T likewise      [C, N] layouts (d on partitions), f32r
  V   = x @ Wv^T + bv                     [N, C] layout (seq on partitions), bf16
  S   = maskneg + Q K^T   (maskneg = (mask-1)*1e9 accumulated into PSUM via an
        identity-matmul; Q K^T via lhsT=Q^T-slice, rhs=K^T-slice)      [n, m]
  T   = S + bias[h]                       (DVE, in-place on the bias tile)
  E, rowsum = exp(0.125 * T)              (ACT with accum_out; masked entries
                                           underflow to exactly 0)
  En  = E * (1/rowsum)  -> bf16           (DVE tensor_scalar, per-partition)
  E^T = PE transpose-mode blocks of En    (bf16 identity)
  A^T[h] = V[:, h] x E^T                  -> mergedT rows h*64..h*64+63 (f32r)
  out = mergedT^T @ mergedT               (contraction over channels, f32r)
"""

import sys

if "/opt/trn_rl_repo" not in sys.path:
    sys.path.insert(0, "/opt/trn_rl_repo")

import ml_dtypes
import numpy as np

P = 128
N = 1024
C = 512
H = 8
D = 64  # head dim
NT = N // P  # 8 row tiles
CT = C // P  # 4 channel tiles
NCORES = 8

_CACHE = {}


def _build_nc():
    import concourse.mybir as mybir
    import concourse.tile as tile
    from concourse import bacc
    from concourse.masks import make_identity

    f32 = mybir.dt.float32
    f32r = mybir.dt.float32r
    bf16 = mybir.dt.bfloat16
    Act = mybir.ActivationFunctionType

    nc = bacc.Bacc("TRN2", target_bir_lowering=False, debug=False)

    # ---- DRAM parameters (per-core) ----
    xT_d = nc.dram_tensor("xT", [C, N], f32, kind="ExternalInput")
    wqT_d = nc.dram_tensor("wqT", [C, C], f32, kind="ExternalInput")
    wkT_d = nc.dram_tensor("wkT", [C, C], f32, kind="ExternalInput")
    wvT_d = nc.dram_tensor("wvT", [C, C], f32, kind="ExternalInput")
    bqP_d = nc.dram_tensor("bqP", [CT, P], f32, kind="ExternalInput")
    bkP_d = nc.dram_tensor("bkP", [CT, P], f32, kind="ExternalInput")
    bv_d = nc.dram_tensor("bv", [1, C], f32, kind="ExternalInput")
    bias_d = nc.dram_tensor("bias", [H, N, N], f32, kind="ExternalInput")
    mneg_d = nc.dram_tensor("mneg", [N, N], bf16, kind="ExternalInput")
    ones_d = nc.dram_tensor("ones", [1, N], f32, kind="ExternalInput")
    out_d = nc.dram_tensor("out", [N, N], f32, kind="ExternalOutput")

    with tile.TileContext(nc) as tc:
        with (
            tc.tile_pool(name="const", bufs=1) as constp,
            tc.tile_pool(name="pers", bufs=1) as pers,
            tc.tile_pool(name="stream", bufs=2) as stream,
            tc.tile_pool(name="psS", bufs=2, space="PSUM") as psS,
            tc.tile_pool(name="psB", bufs=4, space="PSUM") as psB,
        ):
            ident = constp.tile([P, P], f32)
            make_identity(nc, ident[:])
            ident_b = constp.tile([P, P], bf16)
            nc.vector.tensor_copy(ident_b[:], ident[:])
            ones_row = constp.tile([1, N], f32r)
            nc.sync.dma_start(out=ones_row[:], in_=ones_d[:].bitcast(f32r))

            warm = constp.tile([P, 1], f32)
            nc.scalar.activation(warm[:], ident[:, 0:1], Act.Exp, scale=1.0)

            # ---- persistent SBUF tensors ----
            QT = [pers.tile([P, N], f32r, name=f"QT{i}") for i in range(CT)]
            KT = [pers.tile([P, N], f32r, name=f"KT{i}") for i in range(CT)]
            V = [pers.tile([P, C], f32r, name=f"V{i}") for i in range(NT)]
            mneg = [pers.tile([P, N], bf16, name=f"mneg{i}") for i in range(NT)]
            ET = [pers.tile([P, N], f32r, name=f"ET{i}") for i in range(NT)]
            mergedT = [pers.tile([P, N], f32r, name=f"mergedT{i}") for i in range(CT)]
            bq_sb = [pers.tile([P, 1], f32, name=f"bq{i}") for i in range(CT)]
            bk_sb = [pers.tile([P, 1], f32, name=f"bk{i}") for i in range(CT)]
            bv_sb = pers.tile([1, C], f32r, name="bv_sb")

            # ---- phase 0: load x^T and W^T, compute Q^T, K^T, V ----
            with tc.tile_pool(name="qkv_in", bufs=1) as qkvp:
                xT = [qkvp.tile([P, N], f32r, name=f"xT{i}") for i in range(CT)]
                nc.sync.dma_start(
                    out=xT[0][:], in_=xT_d[0:P, :].bitcast(f32r)
                )


                def load_w(dram, nm):
                    ts = []
                    for i in range(CT):
                        t = qkvp.tile([P, C], f32r, name=f"w{nm}{i}", tag="w", bufs=4)
                        nc.sync.dma_start(
                            out=t[:], in_=dram[i * P : (i + 1) * P, :].bitcast(f32r)
                        )
                        ts.append(t)
                    return ts

                # Q^T, K^T: [c-tile, n-chunk] blocks; kt-outer so the first
                # matmul only needs the first W/x tiles off the DMA queue
                for w_d, dst, b_sb, nm in (
                    (wqT_d, QT, bq_sb, "q"),
                    (wkT_d, KT, bk_sb, "k"),
                ):
                    wT = load_w(w_d, nm)
                    if nm == "q":
                        for i in range(1, CT):
                            nc.sync.dma_start(
                                out=xT[i][:],
                                in_=xT_d[i * P : (i + 1) * P, :].bitcast(f32r),
                            )
                        for i in range(CT):
                            nc.sync.dma_start(
                                out=bq_sb[i][:], in_=bqP_d[i, :].unsqueeze(-1)
                            )
                            nc.sync.dma_start(
                                out=bk_sb[i][:], in_=bkP_d[i, :].unsqueeze(-1)
                            )
                        nc.sync.dma_start(out=bv_sb[:], in_=bv_d[:].bitcast(f32r))
                    for j in range(2):
                        pss = [
                            psB.tile(
                                [P, 512],
                                f32,
                                tag=("blk" if ct < 2 else "tpb"),
                                bufs=2,
                                name=f"qk{nm}{j}{ct}",
                            )
                            for ct in range(CT)
                        ]
                        for kt in range(CT):
                            for ct in range(CT):
                                nc.tensor.matmul(
                                    pss[ct][:],
                                    wT[kt][:, ct * P : (ct + 1) * P],
                                    xT[kt][:, j * 512 : (j + 1) * 512],
                                    start=(kt == 0),
                                    stop=(kt == CT - 1),
                                )
                        for ct in range(CT):
                            nc.scalar.activation(
                                dst[ct][:, j * 512 : (j + 1) * 512],
                                pss[ct][:],
                                Act.Identity,
                                bias=b_sb[ct][:],
                            )
                # V natural: [m-tile, c] blocks; extra K=1 step adds bv
                wvT = load_w(wvT_d, "v")
                for mt in range(NT):
                    ps = psB.tile([P, 512], f32, tag="blk", bufs=2)
                    for kt in range(CT):
                        nc.tensor.matmul(
                            ps[:],
                            xT[kt][:, mt * P : (mt + 1) * P],
                            wvT[kt][:],
                            start=(kt == 0),
                            stop=False,
                        )
                    nc.tensor.matmul(
                        ps[:],
                        ones_row[:, mt * P : (mt + 1) * P],
                        bv_sb[:],
                        start=False,
                        stop=True,
                    )
                    nc.scalar.copy(V[mt][:], ps[:])

            # ---- main loop over heads ----
            for h in range(H):
                qt = QT[h // 2]
                kt_sb = KT[h // 2]
                po = (h % 2) * D  # partition offset of this head's rows
                for half in range(2):
                    en_tiles = []
                    for q in range(4):
                        i = half * 4 + q
                        if h == 0:
                            nc.sync.dma_start(
                                out=mneg[i][:],
                                in_=mneg_d[i * P : (i + 1) * P, :],
                            )
                        S = psS.tile([P, N], f32, tag="S")
                        for j in range(2):
                            # maskneg into PSUM via identity-matmul, then QK^T
                            # accumulates on top
                            nc.tensor.matmul(
                                S[:, j * 512 : (j + 1) * 512],
                                ident_b[:],
                                mneg[i][:, j * 512 : (j + 1) * 512],
                                start=True,
                                stop=False,
                            )
                        for j in range(2):
                            nc.tensor.matmul(
                                S[:, j * 512 : (j + 1) * 512],
                                qt[po : po + D, i * P : (i + 1) * P],
                                kt_sb[po : po + D, j * 512 : (j + 1) * 512],
                                start=False,
                                stop=True,
                            )
                        bias_t = stream.tile([P, N], f32, tag="bias", bufs=3)
                        nc.sync.dma_start(
                            out=bias_t[:], in_=bias_d[h, i * P : (i + 1) * P, :]
                        )
                        # in-place: T = S_psum + bias  (overwrites bias tile)
                        nc.vector.tensor_add(bias_t[:], S[:], bias_t[:])
                        E_act = stream.tile([P, N], f32, tag="Eact", bufs=3)
                        rs = stream.tile([P, 1], f32, tag=f"rs{q}", bufs=2)
                        # E = exp(T/8); masked entries underflow to exactly 0,
                        # so accum_out is the masked row-sum
                        nc.scalar.activation(
                            E_act[:], bias_t[:], Act.Exp, scale=0.125, accum_out=rs[:]
                        )
                        rc = stream.tile([P, 1], f32, tag=f"rc{q}", bufs=2)
                        nc.vector.reciprocal(rc[:], rs[:])
                        E_n = stream.tile([P, N], f32, tag=f"En{q}", bufs=2)
                        nc.gpsimd.tensor_scalar_mul(E_n[:], E_act[:], rc[:])
                        en_tiles.append(E_n)
                    # transpose normalized weights: E^T[:, half] blocks (bf16)
                    for mb in range(NT):
                        tp = psB.tile([P, 512], f32, tag="tpb", bufs=2)
                        for q in range(4):
                            nc.tensor.transpose(
                                tp[:, q * P : (q + 1) * P],
                                en_tiles[q][:, mb * P : (mb + 1) * P],
                                ident[:],
                            )
                        cp_dst = ET[mb][:, half * 512 : (half + 1) * 512]
                        if mb % 2 == 0:
                            nc.scalar.copy(cp_dst, tp[:])
                        else:
                            nc.vector.tensor_copy(cp_dst, tp[:])
                # A^T for this head -> mergedT rows
                for j in range(2):
                    Aps = psB.tile([P, 512], f32, tag="blk", bufs=2)
                    for mt in range(NT):
                        nc.tensor.matmul(
                            Aps[0:D, :],
                            V[mt][:, h * D : (h + 1) * D],
                            ET[mt][:, j * 512 : (j + 1) * 512],
                            start=(mt == 0),
                            stop=(mt == NT - 1),
                        )
                    nc.vector.tensor_copy(
                        mergedT[h // 2][po : po + D, j * 512 : (j + 1) * 512],
                        Aps[0:D, :],
                    )

            # ---- final node-similarity GEMM ----
            for i in range(NT):
                o_sb = stream.tile([P, N], f32, tag="o_sb", bufs=2)
                for j in range(2):
                    ps = psB.tile([P, 512], f32, tag="blk", bufs=2)
                    for ct in range(CT):
                        nc.tensor.matmul(
                            ps[:],
                            mergedT[ct][:, i * P : (i + 1) * P],
                            mergedT[ct][:, j * 512 : (j + 1) * 512],
                            start=(ct == 0),
                            stop=(ct == CT - 1),
                        )
                    if j == 0:
                        nc.scalar.copy(o_sb[:, 0:512], ps[:])
                    else:
                        nc.vector.tensor_copy(o_sb[:, 512:1024], ps[:])
                nc.sync.dma_start(out=out_d[i * P : (i + 1) * P, :], in_=o_sb[:])

    nc.compile()
    return nc


def _get_nc():
    if "nc" not in _CACHE:
        _CACHE["nc"] = _build_nc()
    return _CACHE["nc"]


def make_in_maps(inputs):
    x = np.asarray(inputs["x"], dtype=np.float32)
    bias = np.asarray(inputs["bias"], dtype=np.float32)
    mask = np.asarray(inputs["mask"])
    Wq = np.asarray(inputs["Wq"], dtype=np.float32)
    bq = np.asarray(inputs["bq"], dtype=np.float32)
    Wk = np.asarray(inputs["Wk"], dtype=np.float32)
    bk = np.asarray(inputs["bk"], dtype=np.float32)
    Wv = np.asarray(inputs["Wv"], dtype=np.float32)
    bv = np.asarray(inputs["bv"], dtype=np.float32)

    wqT = np.ascontiguousarray(Wq.T)
    wkT = np.ascontiguousarray(Wk.T)
    wvT = np.ascontiguousarray(Wv.T)
    bqP = np.ascontiguousarray(bq.reshape(CT, P))
    bkP = np.ascontiguousarray(bk.reshape(CT, P))
    bvR = np.ascontiguousarray(bv.reshape(1, C))

    in_maps = []
    for b in range(NCORES):
        in_maps.append(
            {
                "xT": np.ascontiguousarray(x[b].T),
                "wqT": wqT,
                "wkT": wkT,
                "wvT": wvT,
                "bqP": bqP,
                "bkP": bkP,
                "bv": bvR,
                "bias": np.ascontiguousarray(bias[b]),
                "mneg": ((mask[b].astype(np.float32) - 1.0) * (2.0**30)).astype(ml_dtypes.bfloat16),
                "ones": np.ones((1, N), np.float32),
            }
        )
    return in_maps


def run(inputs, trace=False, **kw):
    """Run the SPMD kernel; returns (output [8,1024,1024], BassKernelResults)."""
    from concourse.bass_utils import run_bass_kernel_spmd

    nc = _get_nc()
    in_maps = make_in_maps(inputs)
    res = run_bass_kernel_spmd(
        nc, in_maps, core_ids=list(range(NCORES)), trace=trace, **kw
    )
    out = np.stack([res.results[i]["out"] for i in range(NCORES)], axis=0)
    return out, res


def kernel(**inputs):
    out, _ = run(inputs)
    return out
